# revision 54
# baseline (speedup 1.0000x reference)
"""Trainium2 Bass kernel for nn_AttentionBlock (B=16, C=256, H=W=32, 4 heads, d_k=64).

Strategy: data-parallel over batch across 8 NeuronCores (2 batch elements per
core), zero collectives. Per batch element everything is computed in a
"transposed" layout so no on-chip transposes are needed:

  x_b            [C=256, N=1024]        (DRAM layout of x[b], used directly)
  Q_all, K_all   [hd=256, N]  rows h*64+d   (Q has bias+1/8 scale folded in)
  VT             [N, 512]     per head h: cols h*128..+63 = 1.0 (denominator
                              trick), cols h*128+64..+127 = V^T values
  S^T (per head, per j-tile) [j=128, i=1024] = Kh^T·q_i  -> exp on ACT
  AV   (per head)            [128, i]: rows 0-63 = softmax denominator already
                              broadcast 64x, rows 64-127 = unnormalized out^T
  out  = (AV[64:128] * 1/denom) -> [hd, i] -> res^T = Wo @ out + bo_eff + x_b

Softmax max-subtraction is skipped: scores are ~N(0, 0.33) so exp() is safe in
fp32, and softmax is shift-invariant (matches the reference mathematically).
K-bias drops out of softmax (constant over keys j per query); V-bias is folded
into bo_eff = bo + Wo @ bv on the host; the 1/8 scale is folded into Wq/bq.

Matmul operands are bf16 (full PE rate; fp32 would be 4x slower) with fp32
PSUM accumulation; the residual/bias path stays fp32. All PSUM->SBUF
evacuation runs on DVE to keep ACT free for exp, the bottleneck engine.
"""

import os
import numpy as np

NB = 2  # batch elements per core
C = 256
N = 1024
NH = 4
DK = 64
NCORES = 8

_CACHE = {}
LAST_RESULTS = None  # BassKernelResults of the most recent run (for test.py)


def _emit(ctx, tc, nc, tensors):
    from concourse import mybir

    f32 = mybir.dt.float32
    bf16 = mybir.dt.bfloat16
    Exp = mybir.ActivationFunctionType.Exp
    add = mybir.AluOpType.add
    mult = mybir.AluOpType.mult

    x_d, xb_d, wqk_d, wv_d, wo_d, bq_d, bo_d, y_d = tensors

    const = ctx.enter_context(tc.tile_pool(name="const", bufs=1))
    xp = ctx.enter_context(tc.tile_pool(name="xp", bufs=2))
    qkp = ctx.enter_context(tc.tile_pool(name="qkp", bufs=2))
    vtp = ctx.enter_context(tc.tile_pool(name="vtp", bufs=2))
    ep = ctx.enter_context(tc.tile_pool(name="ep", bufs=8))
    ofp = ctx.enter_context(tc.tile_pool(name="ofp", bufs=2))
    avup = ctx.enter_context(tc.tile_pool(name="avup", bufs=4))
    rbp = ctx.enter_context(tc.tile_pool(name="rbp", bufs=2))
    resp = ctx.enter_context(tc.tile_pool(name="resp", bufs=3))
    pps = ctx.enter_context(tc.tile_pool(name="pps", bufs=4, space="PSUM"))

    # ---- constants ----
    wqk_sb = []
    wv_sb = []
    wo_sb = []
    for kt in range(2):
        t = const.tile([128, 512], bf16, tag=f"wqk{kt}")
        nc.sync.dma_start(out=t[:], in_=wqk_d[kt * 128 : (kt + 1) * 128, :])
        wqk_sb.append(t)
        t = const.tile([128, 256], bf16, tag=f"wv{kt}")
        nc.sync.dma_start(out=t[:], in_=wv_d[kt * 128 : (kt + 1) * 128, :])
        wv_sb.append(t)
        t = const.tile([128, 256], bf16, tag=f"wo{kt}")
        nc.sync.dma_start(out=t[:], in_=wo_d[kt * 128 : (kt + 1) * 128, :])
        wo_sb.append(t)
    bq_sb = const.tile([128, 2], f32, tag="bq")
    nc.sync.dma_start(out=bq_sb[:], in_=bq_d[:, :])
    bo_sb = const.tile([128, 2], f32, tag="bo")
    nc.sync.dma_start(out=bo_sb[:], in_=bo_d[:, :])

    # ---- DMAs: bf16 activations first (unblock compute), f32 residual later
    x_sb = {}
    xb_sb = {}
    qk_sb = {}
    vt_sb = {}
    for b in range(NB):
        xbs = []
        for kt in range(2):
            t = xp.tile([128, N], bf16, tag=f"xb{kt}")
            nc.sync.dma_start(out=t[:], in_=xb_d[b, kt * 128 : (kt + 1) * 128, :])
            xbs.append(t)
        xb_sb[b] = xbs
    for b in range(NB):
        xs = []
        for kt in range(2):
            t = xp.tile([128, N], f32, tag=f"x{kt}")
            nc.sync.dma_start(out=t[:], in_=x_d[b, kt * 128 : (kt + 1) * 128, :])
            xs.append(t)
        x_sb[b] = xs

    # ---- per-batch projections ----
    def project(b):
        xbs = xb_sb[b]

        # QK projection: mt 0,1 -> Q head-pairs; mt 2,3 -> K head-pairs
        qks = []
        for mt in range(4):
            ps = pps.tile([128, N], f32, tag="ps")
            for ic in range(2):
                for kt in range(2):
                    nc.tensor.matmul(
                        ps[:, ic * 512 : (ic + 1) * 512],
                        lhsT=wqk_sb[kt][:, mt * 128 : (mt + 1) * 128],
                        rhs=xbs[kt][:, ic * 512 : (ic + 1) * 512],
                        start=(kt == 0),
                        stop=(kt == 1),
                    )
            sb = qkp.tile([128, N], bf16, name=f"qk{b}{mt}", tag=f"qk{mt}")
            if mt < 2:
                nc.vector.tensor_scalar_add(sb[:], ps[:], bq_sb[:, mt : mt + 1])
            else:
                nc.vector.tensor_copy(out=sb[:], in_=ps[:])
            qks.append(sb)
        qk_sb[b] = qks

        # V^T projection. vt tile layout per head h: cols h*128..h*128+63 are
        # 1.0, cols h*128+64..h*128+127 are V^T values. The AV matmul then uses
        # lhsT = vt[:, h*128:(h+1)*128] so its PSUM rows 0-63 come out as the
        # softmax denominator already broadcast across 64 partitions (rows
        # 64-127 are the unnormalized output). The denominator must sit at
        # partition 0: reciprocal_approx_fast (a custom DVE op) misreads
        # inputs that start at a nonzero partition offset.
        vts = []
        for jt in range(8):
            ps = pps.tile([128, 256], f32, tag="ps")
            for kt in range(2):
                nc.tensor.matmul(
                    ps[:],
                    lhsT=xbs[kt][:, jt * 128 : (jt + 1) * 128],
                    rhs=wv_sb[kt][:],
                    start=(kt == 0),
                    stop=(kt == 1),
                )
            sb = vtp.tile([128, 512], bf16, name=f"vt{b}{jt}", tag=f"vt{jt}")
            sbv = sb.rearrange("p (h q) -> p h q", h=4)
            nc.vector.memset(sbv[:, :, 0:64], 1.0)
            nc.vector.tensor_copy(
                out=sbv[:, :, 64:128],
                in_=ps.rearrange("p (h e) -> p h e", h=4)[:, :, :],
            )
            vts.append(sb)
        vt_sb[b] = vts

    # ---- attention: interleave the two batches' head streams for PE density
    of_tiles = {
        b: [
            ofp.tile([128, N], bf16, name=f"of{b}{pr}", tag=f"of{b}{pr}")
            for pr in range(2)
        ]
        for b in range(NB)
    }
    avu_tiles = {}

    def head_stream(b, h):
        pr, hh = divmod(h, 2)
        q_t = qk_sb[b][pr]
        k_t = qk_sb[b][2 + pr]
        # AV accumulator: rows 0-63 the denominator broadcast 64x (from the
        # ones columns of the vt tile), rows 64-127 unnormalized out^T
        av = pps.tile([128, N], f32, name=f"av{b}{h}", tag="ps")
        for jt in range(8):
            st = pps.tile([128, N], f32, name=f"st{b}{h}", tag="ps")
            for ic in range(2):
                nc.tensor.matmul(
                    st[:, ic * 512 : (ic + 1) * 512],
                    lhsT=k_t[hh * 64 : (hh + 1) * 64, jt * 128 : (jt + 1) * 128],
                    rhs=q_t[hh * 64 : (hh + 1) * 64, ic * 512 : (ic + 1) * 512],
                    start=True,
                    stop=True,
                )
            e = ep.tile([128, N], bf16, name=f"e{b}{h}", tag="e")
            nc.scalar.activation(e[:], st[:], Exp)
            for ic in range(2):
                nc.tensor.matmul(
                    av[:, ic * 512 : (ic + 1) * 512],
                    lhsT=vt_sb[b][jt][:, h * 128 : (h + 1) * 128],
                    rhs=e[:, ic * 512 : (ic + 1) * 512],
                    start=(jt == 0),
                    stop=(jt == 7),
                )
        # two short DVE ops release the PSUM accumulator, then normalize
        avu = avup.tile([64, N], bf16, name=f"avu{b}{h}", tag="avu")
        nc.vector.tensor_copy(out=avu[:], in_=av[64:128, :])
        rb = rbp.tile([64, N], f32, name=f"rb{b}{h}", tag="rb")
        nc.vector.reciprocal_approx_fast(out=rb[:], in_=av[0:64, :])
        nc.vector.tensor_tensor(
            out=of_tiles[b][pr][hh * 64 : (hh + 1) * 64, :],
            in0=avu[:],
            in1=rb[:],
            op=mult,
        )

    project(0)
    project(1)
    for h in range(NH):
        for b in range(NB):
            head_stream(b, h)

    # ---- output projection + bias + residual ----
    for b in range(NB):
        for ct in range(2):
            ps = pps.tile([128, N], f32, tag="ps")
            for ic in range(2):
                for kt in range(2):
                    nc.tensor.matmul(
                        ps[:, ic * 512 : (ic + 1) * 512],
                        lhsT=wo_sb[kt][:, ct * 128 : (ct + 1) * 128],
                        rhs=of_tiles[b][kt][:, ic * 512 : (ic + 1) * 512],
                        start=(kt == 0),
                        stop=(kt == 1),
                    )
            res = resp.tile([128, N], f32, tag="res")
            nc.vector.scalar_tensor_tensor(
                out=res[:],
                in0=ps[:],
                scalar=bo_sb[:, ct : ct + 1],
                in1=x_sb[b][ct][:],
                op0=add,
                op1=add,
            )
            nc.sync.dma_start(out=y_d[b, ct * 128 : (ct + 1) * 128, :], in_=res[:])


def _build():
    from contextlib import ExitStack

    import concourse.bacc as bacc
    import concourse.tile as tile
    from concourse import mybir

    f32 = mybir.dt.float32
    bf16 = mybir.dt.bfloat16

    nc = bacc.Bacc(None, target_bir_lowering=False, debug=False)
    x_d = nc.dram_tensor("x", [NB, C, N], f32, kind="ExternalInput")
    xb_d = nc.dram_tensor("xb", [NB, C, N], bf16, kind="ExternalInput")
    wqk_d = nc.dram_tensor("wqkT", [C, 512], bf16, kind="ExternalInput")
    wv_d = nc.dram_tensor("wvT", [C, 256], bf16, kind="ExternalInput")
    wo_d = nc.dram_tensor("woT", [C, 256], bf16, kind="ExternalInput")
    bq_d = nc.dram_tensor("bq2", [128, 2], f32, kind="ExternalInput")
    bo_d = nc.dram_tensor("bo2", [128, 2], f32, kind="ExternalInput")
    y_d = nc.dram_tensor("y", [NB, C, N], f32, kind="ExternalOutput")

    with tile.TileContext(nc) as tc:
        with ExitStack() as ctx:
            _emit(ctx, tc, nc, (x_d, xb_d, wqk_d, wv_d, wo_d, bq_d, bo_d, y_d))
    nc.compile()
    return nc


def _prep_weights(Wp, bp, Wo, bo):
    """Host-side reshuffle of the projection weights into device layouts."""
    import ml_dtypes

    Wp = np.asarray(Wp, np.float32)
    bp = np.asarray(bp, np.float32)
    Wo = np.asarray(Wo, np.float32)
    bo = np.asarray(bo, np.float32)

    idx = np.arange(NH * DK)
    h, d = idx // DK, idx % DK
    q_rows = h * 3 * DK + d
    k_rows = h * 3 * DK + DK + d
    v_rows = h * 3 * DK + 2 * DK + d

    scale = 1.0 / np.sqrt(DK)
    Wq = Wp[q_rows] * scale          # [256, 256], bias+scale folded
    bq = bp[q_rows] * scale
    Wk = Wp[k_rows]                  # K bias drops out of softmax
    Wv = Wp[v_rows]
    bv = bp[v_rows]

    bf16 = ml_dtypes.bfloat16
    wqkT = np.concatenate([Wq, Wk], axis=0).T.astype(bf16)  # [256, 512]
    wvT = np.ascontiguousarray(Wv.T).astype(bf16)           # [256, 256]
    woT = Wo.T.astype(bf16)          # [256, 256]
    bo_eff = (bo + Wo @ bv).astype(np.float32)
    bq2 = np.ascontiguousarray(bq.reshape(2, 128).T)       # [128, 2]
    bo2 = np.ascontiguousarray(bo_eff.reshape(2, 128).T)
    return wqkT, wvT, woT, bq2, bo2


def _install_ntff_hook():
    """Register the axon NTFF profile hook (the image's antenv lacks axon_hooks)."""
    import sys
    import types

    if "antenv.axon_hooks" in sys.modules:
        return
    import antenv

    mod = types.ModuleType("antenv.axon_hooks")
    _state = {"hook": None}
    mod.set_axon_ntff_profile_hook = lambda h: _state.__setitem__("hook", h)
    mod.get_axon_ntff_profile_hook = lambda: _state["hook"]
    sys.modules["antenv.axon_hooks"] = mod
    antenv.axon_hooks = mod
    try:
        from trn_agent_boot.trn_boot import _ntff_profile_via_ctypes

        mod.set_axon_ntff_profile_hook(
            _ntff_profile_via_ctypes("/opt/axon/libaxon_pjrt.so")
        )
    except Exception:
        pass


def kernel(x, Wp, bp, Wo, bo):
    global LAST_RESULTS
    import ml_dtypes
    from concourse.bass_utils import run_bass_kernel_spmd

    x = np.asarray(x, np.float32)
    B, c, hh, ww = x.shape
    assert (B, c, hh * ww) == (NB * NCORES, C, N)
    wqkT, wvT, woT, bq2, bo2 = _prep_weights(Wp, bp, Wo, bo)

    if "nc" not in _CACHE:
        _CACHE["nc"] = _build()
    nc = _CACHE["nc"]

    xf = x.reshape(B, C, N)
    xb = xf.astype(ml_dtypes.bfloat16)
    in_maps = []
    for i in range(NCORES):
        in_maps.append(
            {
                "x": np.ascontiguousarray(xf[i * NB : (i + 1) * NB]),
                "xb": np.ascontiguousarray(xb[i * NB : (i + 1) * NB]),
                "wqkT": wqkT,
                "wvT": wvT,
                "woT": woT,
                "bq2": bq2,
                "bo2": bo2,
            }
        )

    trace = bool(int(os.environ.get("ATTN_KERNEL_TRACE", "0")))
    if trace:
        _install_ntff_hook()
    res = run_bass_kernel_spmd(
        nc, in_maps, list(range(NCORES)), trace=trace
    )
    LAST_RESULTS = res
    y = np.concatenate([res.results[i]["y"] for i in range(NCORES)], axis=0)
    return y.reshape(B, C, hh, ww).astype(np.float32)


# revision 55
# speedup vs baseline: 1.2302x; 1.2302x over previous
"""Trainium2 Bass kernel for nn_AttentionBlock (B=16, C=256, H=W=32, 4 heads, d_k=64).

Strategy: data-parallel over batch across 8 NeuronCores (2 batch elements per
core), zero collectives. Per batch element everything is computed in a
"transposed" layout so no on-chip transposes are needed:

  x_b            [C=256, N=1024]        (DRAM layout of x[b], used directly)
  Q_all, K_all   [hd=256, N]  rows h*64+d   (Q has bias+1/8 scale folded in)
  VT             [N, 512]     per head h: cols h*128..+63 = 1.0 (denominator
                              trick), cols h*128+64..+127 = V^T values
  S^T (per head, per j-tile) [j=128, i=1024] = Kh^T·q_i  -> exp on ACT
  AV   (per head)            [128, i]: rows 0-63 = softmax denominator already
                              broadcast 64x, rows 64-127 = unnormalized out^T
  out  = (AV[64:128] * 1/denom) -> [hd, i] -> res^T = Wo @ out + bo_eff + x_b

Softmax max-subtraction is skipped: scores are ~N(0, 0.33) so exp() is safe in
fp32, and softmax is shift-invariant (matches the reference mathematically).
K-bias drops out of softmax (constant over keys j per query); V-bias is folded
into bo_eff = bo + Wo @ bv on the host; the 1/8 scale is folded into Wq/bq.

Matmul operands are bf16 (full PE rate; fp32 would be 4x slower) with fp32
PSUM accumulation; the residual/bias path stays fp32. All PSUM->SBUF
evacuation runs on DVE to keep ACT free for exp, the bottleneck engine.
"""

import os
import numpy as np

NB = 2  # batch elements per core
C = 256
N = 1024
NH = 4
DK = 64
NCORES = 8

_CACHE = {}
LAST_RESULTS = None  # BassKernelResults of the most recent run (for test.py)


def _emit(ctx, tc, nc, tensors):
    from concourse import mybir

    f32 = mybir.dt.float32
    bf16 = mybir.dt.bfloat16
    Exp = mybir.ActivationFunctionType.Exp
    add = mybir.AluOpType.add
    mult = mybir.AluOpType.mult

    x_d, xb_d, wqk_d, wv_d, wo_d, bq_d, bo_d, y_d = tensors

    const = ctx.enter_context(tc.tile_pool(name="const", bufs=1))
    xp = ctx.enter_context(tc.tile_pool(name="xp", bufs=2))
    qkp = ctx.enter_context(tc.tile_pool(name="qkp", bufs=2))
    vtp = ctx.enter_context(tc.tile_pool(name="vtp", bufs=2))
    ep = ctx.enter_context(tc.tile_pool(name="ep", bufs=6))
    ofp = ctx.enter_context(tc.tile_pool(name="ofp", bufs=2))
    avup = ctx.enter_context(tc.tile_pool(name="avup", bufs=4))
    rbp = ctx.enter_context(tc.tile_pool(name="rbp", bufs=2))
    resp = ctx.enter_context(tc.tile_pool(name="resp", bufs=3))
    pps = ctx.enter_context(tc.tile_pool(name="pps", bufs=4, space="PSUM"))

    # ---- constants ----
    wqk_sb = []
    wv_sb = []
    wo_sb = []
    for kt in range(2):
        t = const.tile([128, 512], bf16, tag=f"wqk{kt}")
        nc.sync.dma_start(out=t[:], in_=wqk_d[kt * 128 : (kt + 1) * 128, :])
        wqk_sb.append(t)
        t = const.tile([128, 256], bf16, tag=f"wv{kt}")
        nc.sync.dma_start(out=t[:], in_=wv_d[kt * 128 : (kt + 1) * 128, :])
        wv_sb.append(t)
        t = const.tile([128, 256], bf16, tag=f"wo{kt}")
        nc.sync.dma_start(out=t[:], in_=wo_d[kt * 128 : (kt + 1) * 128, :])
        wo_sb.append(t)
    bq_sb = const.tile([128, 2], f32, tag="bq")
    nc.sync.dma_start(out=bq_sb[:], in_=bq_d[:, :])
    bo_sb = const.tile([128, 2], f32, tag="bo")
    nc.sync.dma_start(out=bo_sb[:], in_=bo_d[:, :])

    # ---- DMAs: bf16 activations first (unblock compute), f32 residual later
    x_sb = {}
    xb_sb = {}
    qk_sb = {}
    vt_sb = {}
    for b in range(NB):
        xbs = []
        for kt in range(2):
            t = xp.tile([128, N], bf16, tag=f"xb{kt}")
            nc.sync.dma_start(out=t[:], in_=xb_d[b, kt * 128 : (kt + 1) * 128, :])
            xbs.append(t)
        xb_sb[b] = xbs
    for b in range(NB):
        xs = []
        for kt in range(2):
            t = xp.tile([128, N], f32, tag=f"x{kt}")
            nc.sync.dma_start(out=t[:], in_=x_d[b, kt * 128 : (kt + 1) * 128, :])
            xs.append(t)
        x_sb[b] = xs

    # ---- per-batch projections ----
    def project(b):
        xbs = xb_sb[b]

        # QK projection: mt 0,1 -> Q head-pairs; mt 2,3 -> K head-pairs
        qks = []
        for mt in range(4):
            ps = pps.tile([128, N], f32, tag="ps")
            for ic in range(2):
                for kt in range(2):
                    nc.tensor.matmul(
                        ps[:, ic * 512 : (ic + 1) * 512],
                        lhsT=wqk_sb[kt][:, mt * 128 : (mt + 1) * 128],
                        rhs=xbs[kt][:, ic * 512 : (ic + 1) * 512],
                        start=(kt == 0),
                        stop=(kt == 1),
                    )
            sb = qkp.tile([128, N], bf16, name=f"qk{b}{mt}", tag=f"qk{mt}")
            if mt < 2:
                nc.vector.tensor_scalar_add(sb[:], ps[:], bq_sb[:, mt : mt + 1])
            else:
                nc.vector.tensor_copy(out=sb[:], in_=ps[:])
            qks.append(sb)
        qk_sb[b] = qks

        # V^T projection. vt tile layout per head h: cols h*128..h*128+63 are
        # 1.0, cols h*128+64..h*128+127 are V^T values. The AV matmul then uses
        # lhsT = vt[:, h*128:(h+1)*128] so its PSUM rows 0-63 come out as the
        # softmax denominator already broadcast across 64 partitions (rows
        # 64-127 are the unnormalized output). The denominator must sit at
        # partition 0: reciprocal_approx_fast (a custom DVE op) misreads
        # inputs that start at a nonzero partition offset.
        vts = []
        for jt in range(8):
            ps = pps.tile([128, 256], f32, tag="ps")
            for kt in range(2):
                nc.tensor.matmul(
                    ps[:],
                    lhsT=xbs[kt][:, jt * 128 : (jt + 1) * 128],
                    rhs=wv_sb[kt][:],
                    start=(kt == 0),
                    stop=(kt == 1),
                )
            sb = vtp.tile([128, 512], bf16, name=f"vt{b}{jt}", tag=f"vt{jt}")
            sbv = sb.rearrange("p (h q) -> p h q", h=4)
            nc.vector.memset(sbv[:, :, 0:64], 1.0)
            nc.vector.tensor_copy(
                out=sbv[:, :, 64:128],
                in_=ps.rearrange("p (h e) -> p h e", h=4)[:, :, :],
            )
            vts.append(sb)
        vt_sb[b] = vts

    # ---- attention: interleave the two batches' head streams for PE density
    of_tiles = {
        b: [
            ofp.tile([128, N], bf16, name=f"of{b}{pr}", tag=f"of{b}{pr}")
            for pr in range(2)
        ]
        for b in range(NB)
    }
    avu_tiles = {}

    def head_stream(b, h):
        pr, hh = divmod(h, 2)
        q_t = qk_sb[b][pr]
        k_t = qk_sb[b][2 + pr]
        # AV accumulator: rows 0-63 the denominator broadcast 64x (from the
        # ones columns of the vt tile), rows 64-127 unnormalized out^T
        av = pps.tile([128, N], f32, name=f"av{b}{h}", tag="ps")
        for jt in range(8):
            st = pps.tile([128, N], f32, name=f"st{b}{h}", tag="ps")
            for ic in range(2):
                nc.tensor.matmul(
                    st[:, ic * 512 : (ic + 1) * 512],
                    lhsT=k_t[hh * 64 : (hh + 1) * 64, jt * 128 : (jt + 1) * 128],
                    rhs=q_t[hh * 64 : (hh + 1) * 64, ic * 512 : (ic + 1) * 512],
                    start=True,
                    stop=True,
                )
            e = ep.tile([128, N], bf16, name=f"e{b}{h}", tag="e")
            nc.scalar.activation(e[:], st[:], Exp)
            for ic in range(2):
                nc.tensor.matmul(
                    av[:, ic * 512 : (ic + 1) * 512],
                    lhsT=vt_sb[b][jt][:, h * 128 : (h + 1) * 128],
                    rhs=e[:, ic * 512 : (ic + 1) * 512],
                    start=(jt == 0),
                    stop=(jt == 7),
                )
        # two short DVE ops release the PSUM accumulator, then normalize
        avu = avup.tile([64, N], bf16, name=f"avu{b}{h}", tag="avu")
        nc.vector.tensor_copy(out=avu[:], in_=av[64:128, :])
        rb = rbp.tile([64, N], f32, name=f"rb{b}{h}", tag="rb")
        nc.vector.reciprocal_approx_fast(out=rb[:], in_=av[0:64, :])
        nc.vector.tensor_tensor(
            out=of_tiles[b][pr][hh * 64 : (hh + 1) * 64, :],
            in0=avu[:],
            in1=rb[:],
            op=mult,
        )

    project(0)
    project(1)
    for h in range(NH):
        for b in range(NB):
            head_stream(b, h)

    # ---- output projection + bias + residual ----
    for b in range(NB):
        for ct in range(2):
            ps = pps.tile([128, N], f32, tag="ps")
            for ic in range(2):
                for kt in range(2):
                    nc.tensor.matmul(
                        ps[:, ic * 512 : (ic + 1) * 512],
                        lhsT=wo_sb[kt][:, ct * 128 : (ct + 1) * 128],
                        rhs=of_tiles[b][kt][:, ic * 512 : (ic + 1) * 512],
                        start=(kt == 0),
                        stop=(kt == 1),
                    )
            res = resp.tile([128, N], f32, tag="res")
            nc.vector.scalar_tensor_tensor(
                out=res[:],
                in0=ps[:],
                scalar=bo_sb[:, ct : ct + 1],
                in1=x_sb[b][ct][:],
                op0=add,
                op1=add,
            )
            nc.sync.dma_start(out=y_d[b, ct * 128 : (ct + 1) * 128, :], in_=res[:])


def _build():
    from contextlib import ExitStack

    import concourse.bacc as bacc
    import concourse.tile as tile
    from concourse import mybir

    f32 = mybir.dt.float32
    bf16 = mybir.dt.bfloat16

    nc = bacc.Bacc(None, target_bir_lowering=False, debug=False)
    x_d = nc.dram_tensor("x", [NB, C, N], f32, kind="ExternalInput")
    xb_d = nc.dram_tensor("xb", [NB, C, N], bf16, kind="ExternalInput")
    wqk_d = nc.dram_tensor("wqkT", [C, 512], bf16, kind="ExternalInput")
    wv_d = nc.dram_tensor("wvT", [C, 256], bf16, kind="ExternalInput")
    wo_d = nc.dram_tensor("woT", [C, 256], bf16, kind="ExternalInput")
    bq_d = nc.dram_tensor("bq2", [128, 2], f32, kind="ExternalInput")
    bo_d = nc.dram_tensor("bo2", [128, 2], f32, kind="ExternalInput")
    y_d = nc.dram_tensor("y", [NB, C, N], f32, kind="ExternalOutput")

    with tile.TileContext(nc) as tc:
        with ExitStack() as ctx:
            _emit(ctx, tc, nc, (x_d, xb_d, wqk_d, wv_d, wo_d, bq_d, bo_d, y_d))
    nc.compile()
    return nc


def _prep_weights(Wp, bp, Wo, bo):
    """Host-side reshuffle of the projection weights into device layouts."""
    import ml_dtypes

    Wp = np.asarray(Wp, np.float32)
    bp = np.asarray(bp, np.float32)
    Wo = np.asarray(Wo, np.float32)
    bo = np.asarray(bo, np.float32)

    idx = np.arange(NH * DK)
    h, d = idx // DK, idx % DK
    q_rows = h * 3 * DK + d
    k_rows = h * 3 * DK + DK + d
    v_rows = h * 3 * DK + 2 * DK + d

    scale = 1.0 / np.sqrt(DK)
    Wq = Wp[q_rows] * scale          # [256, 256], bias+scale folded
    bq = bp[q_rows] * scale
    Wk = Wp[k_rows]                  # K bias drops out of softmax
    Wv = Wp[v_rows]
    bv = bp[v_rows]

    bf16 = ml_dtypes.bfloat16
    wqkT = np.concatenate([Wq, Wk], axis=0).T.astype(bf16)  # [256, 512]
    wvT = np.ascontiguousarray(Wv.T).astype(bf16)           # [256, 256]
    woT = Wo.T.astype(bf16)          # [256, 256]
    bo_eff = (bo + Wo @ bv).astype(np.float32)
    bq2 = np.ascontiguousarray(bq.reshape(2, 128).T)       # [128, 2]
    bo2 = np.ascontiguousarray(bo_eff.reshape(2, 128).T)
    return wqkT, wvT, woT, bq2, bo2


def _install_ntff_hook():
    """Register the axon NTFF profile hook (the image's antenv lacks axon_hooks)."""
    import sys
    import types

    if "antenv.axon_hooks" in sys.modules:
        return
    import antenv

    mod = types.ModuleType("antenv.axon_hooks")
    _state = {"hook": None}
    mod.set_axon_ntff_profile_hook = lambda h: _state.__setitem__("hook", h)
    mod.get_axon_ntff_profile_hook = lambda: _state["hook"]
    sys.modules["antenv.axon_hooks"] = mod
    antenv.axon_hooks = mod
    try:
        from trn_agent_boot.trn_boot import _ntff_profile_via_ctypes

        mod.set_axon_ntff_profile_hook(
            _ntff_profile_via_ctypes("/opt/axon/libaxon_pjrt.so")
        )
    except Exception:
        pass


def kernel(x, Wp, bp, Wo, bo):
    global LAST_RESULTS
    import ml_dtypes
    from concourse.bass_utils import run_bass_kernel_spmd

    x = np.asarray(x, np.float32)
    B, c, hh, ww = x.shape
    assert (B, c, hh * ww) == (NB * NCORES, C, N)
    wqkT, wvT, woT, bq2, bo2 = _prep_weights(Wp, bp, Wo, bo)

    if "nc" not in _CACHE:
        _CACHE["nc"] = _build()
    nc = _CACHE["nc"]

    xf = x.reshape(B, C, N)
    xb = xf.astype(ml_dtypes.bfloat16)
    in_maps = []
    for i in range(NCORES):
        in_maps.append(
            {
                "x": np.ascontiguousarray(xf[i * NB : (i + 1) * NB]),
                "xb": np.ascontiguousarray(xb[i * NB : (i + 1) * NB]),
                "wqkT": wqkT,
                "wvT": wvT,
                "woT": woT,
                "bq2": bq2,
                "bo2": bo2,
            }
        )

    trace = bool(int(os.environ.get("ATTN_KERNEL_TRACE", "0")))
    if trace:
        _install_ntff_hook()
    res = run_bass_kernel_spmd(
        nc, in_maps, list(range(NCORES)), trace=trace
    )
    LAST_RESULTS = res
    y = np.concatenate([res.results[i]["y"] for i in range(NCORES)], axis=0)
    return y.reshape(B, C, hh, ww).astype(np.float32)
